# revision 1
# baseline (speedup 1.0000x reference)
"""FP64->FP32 bit-circuit converter kernel for Trainium2 (8 NeuronCores).

Input:  fp64_pulse (1048576, 64) float32 of {0,1} bits (fp64, MSB first).
Output: (1048576, 32) float32 of {0,1} bits (fp32 conversion result).

Strategy (pure data parallel over batch, 131072 rows/core):
  - batch-2D layout: 128 partitions x 1024 inner rows per core,
    supertiles of NF inner rows ([128, NF*64] input tiles),
  - bit packing into integers via one broadcast-weight multiply and
    innermost-axis reduces (exp_val, mant_int, sticky_sum),
  - mantissa round-to-nearest-even done by fp32 hardware: adding
    frac = 0.5*R + 0.25*S to the 24-bit int (2^23 + mant_int) rounds
    exactly like the reference ripple-adder circuit,
  - special values (nan/inf/overflow/underflow) folded in as value-level
    muxes, then output bits extracted with int32 (shift, and) ops.
"""
import numpy as np

from concourse import bacc, mybir
from concourse.tile import TileContext
from concourse.bass_utils import run_bass_kernel_spmd

AOT = mybir.AluOpType
F32 = mybir.dt.float32
BF16 = mybir.dt.bfloat16
I32 = mybir.dt.int32

B = 1_048_576
N_CORES = 8
B_CORE = B // N_CORES          # 131072
P = 128                        # partitions
NI = B_CORE // P               # 1024 inner rows per partition
NF = 128                       # inner rows per supertile
N_ST = NI // NF                # supertiles per core
D_IN = 64
D_OUT = 32

# weight row: col 0 sign (unused), cols 1..11 exp MSB-first (2^10..2^0),
# cols 12..34 mant bits 0..22 MSB-first (2^22..2^0), rest 0.
_w_row = np.zeros(D_IN, np.float32)
_w_row[1:12] = [2.0 ** (10 - k) for k in range(11)]
_w_row[12:35] = [2.0 ** (22 - k) for k in range(23)]
import ml_dtypes
WCONST = np.broadcast_to(_w_row, (P, D_IN)).astype(ml_dtypes.bfloat16).copy()

_CACHE = {}


def _build():
    nc = bacc.Bacc("TRN2")
    x = nc.dram_tensor("x", [B_CORE, D_IN], F32, kind="ExternalInput")
    w = nc.dram_tensor("w", [P, D_IN], BF16, kind="ExternalInput")
    y = nc.dram_tensor("y", [B_CORE, D_OUT], F32, kind="ExternalOutput")

    x_r = x.ap().rearrange("(p n) d -> p (n d)", p=P)   # [128, NI*64]
    y_r = y.ap().rearrange("(p n) d -> p (n d)", p=P)   # [128, NI*32]

    with TileContext(nc) as tc:
        with (
            tc.tile_pool(name="consts", bufs=1) as consts,
            tc.tile_pool(name="io", bufs=2) as io,
            tc.tile_pool(name="mid", bufs=2) as mid,
            tc.tile_pool(name="sc", bufs=3) as sc,
        ):
            wt = consts.tile([P, D_IN], BF16)
            nc.sync.dma_start(wt[:, :], w.ap())

            # small first/last supertiles shorten the DMA head/tail ramp
            schedule = [32, 96] + [NF] * (N_ST - 2) + [96, 32]
            assert sum(schedule) == NI
            off = 0
            for nf in schedule:
                xin = io.tile([P, nf * D_IN], F32, tag="xin")
                nc.sync.dma_start(
                    xin[:, :], x_r[:, off * D_IN:(off + nf) * D_IN])

                xv = xin[:, :].rearrange("p (n d) -> p n d", d=D_IN)

                # --- weighted pack: mult by broadcast weights + reduces ---
                wb = wt[:, 1:35].unsqueeze(1).broadcast_to([P, nf, 34])
                xw = mid.tile([P, nf * 34], F32, tag="xw")
                xwv = xw[:, :].rearrange("p (n d) -> p n d", d=34)
                nc.vector.tensor_tensor(xwv, xv[:, :, 1:35], wb, AOT.mult)

                exp_val_t = sc.tile([P, nf], F32, tag="exp_val")
                nc.vector.tensor_reduce(
                    exp_val_t[:, :].unsqueeze(2), xwv[:, :, 0:11],
                    mybir.AxisListType.X, AOT.add)
                mant_int_t = sc.tile([P, nf], F32, tag="mant_int")
                nc.vector.tensor_reduce(
                    mant_int_t[:, :].unsqueeze(2), xwv[:, :, 11:34],
                    mybir.AxisListType.X, AOT.add)
                sticky_t = sc.tile([P, nf], F32, tag="sticky")
                nc.vector.tensor_reduce(
                    sticky_t[:, :].unsqueeze(2), xv[:, :, 36:64],
                    mybir.AxisListType.X, AOT.add)
                exp_val = exp_val_t[:, :]
                mant_int = mant_int_t[:, :]
                sticky = sticky_t[:, :]
                Rbit = xv[:, :, 35]
                sign = xv[:, :, 0]

                # --- mantissa RNE via hw fp32 add ---
                fr2 = sc.tile([P, nf], BF16, tag="fr2")
                nc.any.tensor_scalar(fr2[:, :], sticky, 1.0, 0.25,
                                     AOT.is_ge, AOT.mult)
                frac = sc.tile([P, nf], BF16, tag="frac")
                nc.vector.scalar_tensor_tensor(frac[:, :], Rbit, 0.5, fr2[:, :],
                                               AOT.mult, AOT.add)
                Mr = sc.tile([P, nf], F32, tag="Mr")
                nc.vector.scalar_tensor_tensor(Mr[:, :], mant_int,
                                               float(2 ** 23), frac[:, :],
                                               AOT.add, AOT.add)
                c_m = sc.tile([P, nf], BF16, tag="c_m")
                nc.any.tensor_scalar(c_m[:, :], Mr[:, :], float(2 ** 24),
                                     None, AOT.is_ge)
                # Mval = Mr - 2^23*c_m  (in [2^23, 2^24); bit 23 never read)
                Mval = sc.tile([P, nf], F32, tag="Mval")
                nc.vector.scalar_tensor_tensor(Mval[:, :], c_m[:, :],
                                               float(-(2 ** 23)), Mr[:, :],
                                               AOT.mult, AOT.add)

                # --- exponent value T2 = exp_val + c_m + 1152 ---
                T2 = sc.tile([P, nf], F32, tag="T2")
                nc.vector.scalar_tensor_tensor(T2[:, :], c_m[:, :], 1152.0,
                                               exp_val, AOT.add, AOT.add)

                # --- specials ---
                over = sc.tile([P, nf], BF16, tag="over")
                nc.any.tensor_scalar(over[:, :], exp_val, 1151.0, None,
                                     AOT.is_ge)
                under = sc.tile([P, nf], BF16, tag="under")
                nc.any.tensor_scalar(under[:, :], exp_val, 897.0, None,
                                     AOT.is_lt)
                # m_any = (mant_int + 2*frac) >= 0.5  (frac = 0.5R + 0.25S)
                ms1 = sc.tile([P, nf], F32, tag="ms1")
                nc.vector.scalar_tensor_tensor(ms1[:, :], frac[:, :], 2.0,
                                               mant_int, AOT.mult, AOT.add)
                m_any = sc.tile([P, nf], F32, tag="m_any")
                nc.any.tensor_scalar(m_any[:, :], ms1[:, :], 0.5, None, AOT.is_ge)
                is_max = sc.tile([P, nf], BF16, tag="is_max")
                nc.any.tensor_scalar(is_max[:, :], exp_val, 2047.0, None,
                                     AOT.is_ge)
                # nan = is_max AND m_any, via sum >= 2 (fused into nv below)
                nan2 = sc.tile([P, nf], F32, tag="nan2")
                nc.vector.tensor_tensor(nan2[:, :], is_max[:, :], m_any[:, :],
                                        AOT.add)
                ou = sc.tile([P, nf], BF16, tag="ou")
                nc.vector.tensor_tensor(ou[:, :], over[:, :], under[:, :],
                                        AOT.add)
                Acoef = sc.tile([P, nf], BF16, tag="Acoef")
                nc.any.tensor_scalar(Acoef[:, :], ou[:, :], -1.0, 1.0,
                                     AOT.mult, AOT.add)

                # --- value-level muxes ---
                VV = sc.tile([P, 2 * nf], F32, tag="VV")
                Vexp = VV[:, 0:nf]
                Vm = VV[:, nf:2 * nf]
                vx = sc.tile([P, nf], F32, tag="vx")
                nc.vector.tensor_tensor(vx[:, :], T2[:, :], Acoef[:, :],
                                        AOT.mult)
                nc.vector.scalar_tensor_tensor(Vexp, over[:, :], 255.0,
                                               vx[:, :], AOT.mult, AOT.add)
                nv = sc.tile([P, nf], BF16, tag="nv")
                nc.any.tensor_scalar(nv[:, :], nan2[:, :], 2.0, float(2 ** 22),
                                     AOT.is_ge, AOT.mult)
                mx = sc.tile([P, nf], F32, tag="mx")
                nc.vector.tensor_tensor(mx[:, :], Mval[:, :], Acoef[:, :],
                                        AOT.mult)
                nc.vector.tensor_tensor(Vm, mx[:, :], nv[:, :], AOT.add)

                # --- bit extraction (int32); one wide convert for both ---
                VVi = sc.tile([P, 2 * nf], I32, tag="VVi")
                nc.any.tensor_copy(VVi[:, :], VV[:, :])
                Vexpi = VVi[:, 0:nf]
                Vmi = VVi[:, nf:2 * nf]

                yti = mid.tile([P, nf * D_OUT], I32, tag="yti")
                ytv = yti[:, :].rearrange("p (n d) -> p n d", d=D_OUT)
                # sign (col 0): f32 -> i32 convert copy
                nc.any.tensor_copy(ytv[:, :, 0], sign)
                # exp cols 1..8 = T2 bits 7..0
                for c in range(1, 9):
                    j = 8 - c
                    nc.any.tensor_scalar(ytv[:, :, c], Vexpi, j, 1,
                                         AOT.logical_shift_right,
                                         AOT.bitwise_and)
                # mant cols 9..31 = M bits 22..0
                for c in range(9, 32):
                    j = 31 - c
                    nc.any.tensor_scalar(ytv[:, :, c], Vmi, j, 1,
                                         AOT.logical_shift_right,
                                         AOT.bitwise_and)

                # int32 -> f32 convert in place via bitcast view, then DMA out
                ytf = yti[:, :].bitcast(F32)
                nc.any.tensor_copy(ytf, yti[:, :])
                nc.sync.dma_start(
                    y_r[:, off * D_OUT:(off + nf) * D_OUT], ytf)
                off += nf

    nc.compile()
    return nc


def _get_nc():
    if "nc" not in _CACHE:
        _CACHE["nc"] = _build()
    return _CACHE["nc"]


def kernel(fp64_pulse: np.ndarray) -> np.ndarray:
    x = np.ascontiguousarray(fp64_pulse, dtype=np.float32)
    assert x.shape == (B, D_IN)
    nc = _get_nc()
    in_maps = [
        {"x": x[c * B_CORE:(c + 1) * B_CORE], "w": WCONST}
        for c in range(N_CORES)
    ]
    res = run_bass_kernel_spmd(nc, in_maps, core_ids=list(range(N_CORES)))
    return np.concatenate([r["y"] for r in res.results], axis=0)



# revision 2
# speedup vs baseline: 1.9262x; 1.9262x over previous
"""FP64->FP32 bit-circuit converter for Trainium2 (8 NeuronCores), packed I/O.

The end-to-end cost of kernel() is dominated by host<->device transport over
the axon tunnel (~50-150 MB/s, ~140 ms fixed latency per fetch), not device
execution.  Strategy (pure data parallel over the batch, 131072 rows/core):

  host:   pack the (B, 64) {0,1}-float input into 2 int32 words per row
          (8 MB instead of 256 MB) with a multithreaded XLA-CPU jit;
  device: run the whole conversion as ~36 fused int32 ALU ops per row
          (shift/and/or/xor/add/compare) on each core's [128 x 1024] layout,
          emitting the literal IEEE fp32 bit pattern as one int32 per row
          (4 MB back instead of 128 MB);
  host:   expand the words back into the (B, 32) float bit matrix.

Bit layout (MSB-first, matching the column order of the reference):
  hi = row bits 0..31  (bit0=sign at bit31, bits1..11=exp, bits12..31=mant0..19)
  lo = row bits 32..63 (mant bits 20..51)
Output word = sign<<31 | exp8<<23 | mant23 -- exactly the fp32 bit pattern,
whose MSB-first bit expansion equals the 32 output columns.

The Bass kernel is compiled and first executed via
bass_utils.run_bass_kernel_spmd (during warm-up, which also cross-checks the
fast path against it); steady-state calls reuse one cached jit executor to
avoid per-call retracing, with the donated output buffers created on-device
so no zero pages cross the tunnel.  Warm-up starts in a background thread at
import so axon connection + neuronxcc compile overlap harness setup.
"""
import threading
import numpy as np
import jax
import jax.numpy as jnp
from jax.sharding import Mesh, PartitionSpec, NamedSharding
from jax.experimental.shard_map import shard_map

from concourse import bacc, bass2jax, mybir
from concourse.tile import TileContext
from concourse.bass_utils import run_bass_kernel_spmd

AOT = mybir.AluOpType
I32 = mybir.dt.int32

B = 1_048_576
N_CORES = 8
B_CORE = B // N_CORES          # 131072
P = 128                        # SBUF partitions
NI = B_CORE // P               # 1024 rows per partition
D_IN = 2                       # packed words per row
D_OUT = 1


def _build():
    nc = bacc.Bacc("TRN2")
    x = nc.dram_tensor("x", [B_CORE, D_IN], I32, kind="ExternalInput")
    y = nc.dram_tensor("y", [B_CORE, D_OUT], I32, kind="ExternalOutput")

    x_r = x.ap().rearrange("(p n) d -> p (n d)", p=P)   # [128, NI*2]
    y_r = y.ap().rearrange("(p n) d -> p (n d)", p=P)   # [128, NI]

    NF = NI // 2  # two supertiles: overlap in-DMA / compute / out-DMA

    with TileContext(nc) as tc:
        with (
            tc.tile_pool(name="io", bufs=2) as io,
            tc.tile_pool(name="sc", bufs=2) as sc,
        ):
            for st in range(NI // NF):
                off = st * NF
                xin = io.tile([P, NF * D_IN], I32, tag="xin", name="xin")
                nc.sync.dma_start(xin[:, :], x_r[:, off * D_IN:(off + NF) * D_IN])
                xv = xin[:, :].rearrange("p (n d) -> p n d", d=D_IN)
                hi = xv[:, :, 0]
                lo = xv[:, :, 1]

                def T(tag):
                    t = sc.tile([P, NF], I32, tag=tag, name=tag)
                    return t[:, :]

                # field extraction
                E = T("E")          # 11-bit biased fp64 exponent
                nc.any.tensor_scalar(E, hi, 20, 0x7FF,
                                     AOT.logical_shift_right, AOT.bitwise_and)
                Mhi = T("Mhi")
                nc.any.tensor_scalar(Mhi, hi, 0xFFFFF, 3,
                                     AOT.bitwise_and, AOT.logical_shift_left)
                Mlo = T("Mlo")
                nc.any.tensor_scalar(Mlo, lo, 29, 7,
                                     AOT.logical_shift_right, AOT.bitwise_and)
                M23 = T("M23")      # top 23 mantissa bits as an int
                nc.any.tensor_tensor(M23, Mhi, Mlo, AOT.bitwise_or)
                R = T("R")          # round bit (mant bit 23)
                nc.any.tensor_scalar(R, lo, 28, 1,
                                     AOT.logical_shift_right, AOT.bitwise_and)
                sval = T("sval")    # sticky field (mant bits 24..51)
                nc.any.tensor_scalar(sval, lo, 0x0FFFFFFF, None, AOT.bitwise_and)
                S = T("S")
                nc.any.tensor_scalar(S, sval, 1, None, AOT.min)
                # round-to-nearest-even: round_up = R & (S | lsb)
                L = T("L")
                nc.any.tensor_scalar(L, M23, 1, None, AOT.bitwise_and)
                SL = T("SL")
                nc.any.tensor_tensor(SL, S, L, AOT.bitwise_or)
                ru = T("ru")
                nc.any.tensor_tensor(ru, R, SL, AOT.bitwise_and)
                Mr = T("Mr")
                nc.any.tensor_tensor(Mr, M23, ru, AOT.add)
                c_m = T("c_m")      # mantissa carry into the exponent
                nc.any.tensor_scalar(c_m, Mr, 23, None, AOT.logical_shift_right)
                mant_f = T("mant_f")
                nc.any.tensor_scalar(mant_f, Mr, 0x7FFFFF, None, AOT.bitwise_and)
                # rebias: newE = (E - 896) + c_m
                newE = T("newE")
                nc.vector.scalar_tensor_tensor(newE, E, -896, c_m,
                                               AOT.add, AOT.add)
                nsh = T("nsh")
                nc.any.tensor_scalar(nsh, newE, 23, None, AOT.logical_shift_left)
                body = T("body")
                nc.any.tensor_tensor(body, nsh, mant_f, AOT.bitwise_or)
                # specials
                over = T("over")
                nc.any.tensor_scalar(over, E, 1151, None, AOT.is_ge)
                under = T("under")
                nc.any.tensor_scalar(under, E, 897, None, AOT.is_lt)
                lml = T("lml")      # mant bits 23..51
                nc.any.tensor_scalar(lml, lo, 0x1FFFFFFF, None, AOT.bitwise_and)
                manyv = T("manyv")
                nc.any.tensor_tensor(manyv, M23, lml, AOT.bitwise_or)
                eq2047 = T("eq2047")
                nc.any.tensor_scalar(eq2047, E, 2047, None, AOT.is_equal)
                many = T("many")
                nc.any.tensor_scalar(many, manyv, 0, None, AOT.not_equal)
                nan = T("nan")
                nc.any.tensor_tensor(nan, eq2047, many, AOT.bitwise_and)
                # body1 = over ? 0x7F800000 : body   (xor/and with NOT-mask)
                om = T("om")
                nc.any.tensor_scalar(om, over, 1, None, AOT.subtract)
                x1 = T("x1")
                nc.any.tensor_scalar(x1, body, 0x7F800000, None, AOT.bitwise_xor)
                x2 = T("x2")
                nc.any.tensor_tensor(x2, x1, om, AOT.bitwise_and)
                body1 = T("body1")
                nc.any.tensor_scalar(body1, x2, 0x7F800000, None, AOT.bitwise_xor)
                # body2 = under ? 0 : body1
                um = T("um")
                nc.any.tensor_scalar(um, under, 1, None, AOT.subtract)
                body2 = T("body2")
                nc.any.tensor_tensor(body2, body1, um, AOT.bitwise_and)
                # body3 = nan ? 0x7FC00000 : body2
                nm = T("nm")
                nc.any.tensor_scalar(nm, nan, 1, None, AOT.subtract)
                x3 = T("x3")
                nc.any.tensor_scalar(x3, body2, 0x7FC00000, None, AOT.bitwise_xor)
                x4 = T("x4")
                nc.any.tensor_tensor(x4, x3, nm, AOT.bitwise_and)
                body3 = T("body3")
                nc.any.tensor_scalar(body3, x4, 0x7FC00000, None, AOT.bitwise_xor)
                # sign bit stays at bit 31
                sb = T("sb")
                nc.any.tensor_scalar(sb, hi, 31, 31,
                                     AOT.logical_shift_right, AOT.logical_shift_left)
                yt = io.tile([P, NF], I32, tag="yt", name="yt")
                nc.any.tensor_tensor(yt[:, :], body3, sb, AOT.bitwise_or)
                nc.sync.dma_start(y_r[:, off:off + NF], yt[:, :])

    nc.compile()
    return nc


# ---------------- host-side pack / unpack (XLA CPU, multithreaded) ----------
_PACK_W = (np.uint32(1) << np.arange(31, -1, -1, dtype=np.uint32)).astype(np.int32)


def _pack_cpu(xf):
    # {0.,1.} float bits, MSB-first -> int32 words; int32 add-wrap == OR here
    xi = xf.astype(jnp.int32).reshape(-1, D_IN, 32)
    return (xi * _PACK_W[None, None, :]).sum(axis=-1, dtype=jnp.int32)


def _unpack_cpu(w):
    sh = jnp.arange(31, -1, -1, dtype=jnp.int32)
    bits = jnp.right_shift(w.reshape(-1, 1).view(jnp.uint32),
                           sh.view(jnp.uint32)[None, :]) & jnp.uint32(1)
    return bits.astype(jnp.float32)


def _pack_input_np(x: np.ndarray) -> np.ndarray:
    xp = np.packbits(x != 0, axis=-1)
    return xp.view(np.dtype(">u4")).astype(np.uint32).view(np.int32)


def _unpack_output_np(w: np.ndarray) -> np.ndarray:
    wbe = w.view(np.uint32).astype(np.dtype(">u4"))
    bits = np.unpackbits(wbe.view(np.uint8).reshape(-1, 4), axis=-1)
    return bits.astype(np.float32)


# ---------------- cached executor ----------------
_STATE: dict = {}
_LOCK = threading.Lock()


def _prepare_locked():
    if "ready" in _STATE or "failed" in _STATE:
        return
    try:
        nc = _build()
        _STATE["nc"] = nc

        # official path first: compile + run the Bass kernel via
        # run_bass_kernel_spmd (dummy input); also warms devices + NEFF.
        dummy = np.zeros((B_CORE, D_IN), np.int32)
        in_maps = [{"x": dummy} for _ in range(N_CORES)]
        res = run_bass_kernel_spmd(nc, in_maps, core_ids=list(range(N_CORES)))
        w_official = np.concatenate([r["y"] for r in res.results], axis=0)

        pack_jit = jax.jit(_pack_cpu, backend="cpu")
        unpack_jit = jax.jit(_unpack_cpu, backend="cpu")

        bass2jax.install_neuronx_cc_hook()
        pn = nc.partition_id_tensor.name if nc.partition_id_tensor else None
        in_names, out_names, out_avals = [], [], []
        for alloc in nc.m.functions[0].allocations:
            if not isinstance(alloc, mybir.MemoryLocationSet):
                continue
            name = alloc.memorylocations[0].name
            if alloc.kind == "ExternalInput":
                if name != pn:
                    in_names.append(name)
            elif alloc.kind == "ExternalOutput":
                out_names.append(name)
                out_avals.append(jax.core.ShapedArray(
                    tuple(alloc.tensor_shape), mybir.dt.np(alloc.dtype)))
        n_params, n_outs = len(in_names), len(out_avals)
        in_names_all = in_names + out_names + ([pn] if pn else [])
        donate = tuple(range(n_params, n_params + n_outs))

        def _body(*args):
            operands = list(args)
            if pn is not None:
                operands.append(bass2jax.partition_id_tensor())
            return tuple(bass2jax._bass_exec_p.bind(
                *operands, out_avals=tuple(out_avals),
                in_names=tuple(in_names_all), out_names=tuple(out_names),
                lowering_input_output_aliases=(),
                sim_require_finite=True, sim_require_nnan=True, nc=nc))

        devices = jax.devices()[:N_CORES]
        mesh = Mesh(np.asarray(devices), ("core",))
        spec = PartitionSpec("core")
        shd = NamedSharding(mesh, spec)
        sharded = jax.jit(
            shard_map(_body, mesh=mesh, in_specs=(spec,) * (n_params + n_outs),
                      out_specs=(spec,) * n_outs, check_rep=False),
            donate_argnums=donate, keep_unused=True)
        g_out = (N_CORES * out_avals[0].shape[0], *out_avals[0].shape[1:])
        zeros_jit = jax.jit(lambda: jnp.zeros(g_out, out_avals[0].dtype),
                            out_shardings=shd)

        # warm-compile + cross-check the fast path against the official run
        xg = np.zeros((B, D_IN), np.int32)
        out = sharded(xg, zeros_jit())
        w_fast = np.asarray(out[0])
        assert np.array_equal(w_fast, w_official), "fast path mismatch"
        # warm the host pack/unpack jits too
        pack_jit(np.zeros((4096, 64), np.float32))
        unpack_jit(np.zeros((4096, 1), np.int32))

        _STATE.update(dict(pack_jit=pack_jit, unpack_jit=unpack_jit,
                           sharded=sharded, zeros_jit=zeros_jit, ready=True))
    except Exception as e:  # fall back to the plain spmd path per call
        _STATE["failed"] = repr(e)
        if "nc" not in _STATE:
            _STATE["nc"] = _build()


def _prepare():
    with _LOCK:
        _prepare_locked()


def _get_nc():
    _prepare()
    return _STATE["nc"]


_WARM = threading.Thread(target=_prepare, daemon=True)
_WARM.start()


def kernel(fp64_pulse: np.ndarray) -> np.ndarray:
    x = np.asarray(fp64_pulse)
    assert x.shape == (B, 64)
    _prepare()
    if "ready" in _STATE:
        zeros = _STATE["zeros_jit"]()                    # async, on-device
        xw = np.asarray(_STATE["pack_jit"](x))           # (B, 2) int32
        out = _STATE["sharded"](xw, zeros)
        w = np.asarray(out[0])                           # (B, 1) int32
        return np.asarray(_STATE["unpack_jit"](w))
    # fallback: plain official path with numpy pack/unpack
    nc = _STATE["nc"]
    xw = _pack_input_np(x)
    in_maps = [{"x": xw[c * B_CORE:(c + 1) * B_CORE]} for c in range(N_CORES)]
    res = run_bass_kernel_spmd(nc, in_maps, core_ids=list(range(N_CORES)))
    w = np.concatenate([r["y"] for r in res.results], axis=0)
    return _unpack_output_np(w)


# revision 7
# speedup vs baseline: 1.9640x; 1.0196x over previous
"""FP64->FP32 bit-circuit converter for Trainium2 (8 NeuronCores), packed I/O.

The end-to-end cost of kernel() is dominated by host<->device transport over
the axon tunnel (~50-150 MB/s, ~140 ms fixed latency per fetch), not device
execution.  Strategy (pure data parallel over the batch, 131072 rows/core):

  host:   pack the (B, 64) {0,1}-float input into 2 int32 words per row
          (8 MB instead of 256 MB) with a multithreaded XLA-CPU jit;
  device: run the whole conversion as ~36 fused int32 ALU ops per row
          (shift/and/or/xor/add/compare) on each core's [128 x 1024] layout,
          emitting the literal IEEE fp32 bit pattern as one int32 per row
          (4 MB back instead of 128 MB);
  host:   expand the words back into the (B, 32) float bit matrix.

Bit layout (MSB-first, matching the column order of the reference):
  hi = row bits 0..31  (bit0=sign at bit31, bits1..11=exp, bits12..31=mant0..19)
  lo = row bits 32..63 (mant bits 20..51)
Output word = sign<<31 | exp8<<23 | mant23 -- exactly the fp32 bit pattern,
whose MSB-first bit expansion equals the 32 output columns.

The Bass kernel is compiled and first executed via
bass_utils.run_bass_kernel_spmd (during warm-up, which also cross-checks the
fast path against it); steady-state calls reuse one cached jit executor to
avoid per-call retracing, with the donated output buffers created on-device
so no zero pages cross the tunnel.  Warm-up starts in a background thread at
import so axon connection + neuronxcc compile overlap harness setup.
"""
import threading
import numpy as np
import jax
import jax.numpy as jnp
from jax.sharding import Mesh, PartitionSpec, NamedSharding
from jax.experimental.shard_map import shard_map

from concourse import bacc, bass2jax, mybir
from concourse.tile import TileContext
from concourse.bass_utils import run_bass_kernel_spmd

AOT = mybir.AluOpType
I32 = mybir.dt.int32

B = 1_048_576
N_CORES = 8
B_CORE = B // N_CORES          # 131072
P = 128                        # SBUF partitions
NI = B_CORE // P               # 1024 rows per partition
D_IN = 2                       # packed words per row
D_OUT = 1


def _build():
    nc = bacc.Bacc("TRN2")
    x = nc.dram_tensor("x", [B_CORE, D_IN], I32, kind="ExternalInput")
    y = nc.dram_tensor("y", [B_CORE, D_OUT], I32, kind="ExternalOutput")

    x_r = x.ap().rearrange("(p n) d -> p (n d)", p=P)   # [128, NI*2]
    y_r = y.ap().rearrange("(p n) d -> p (n d)", p=P)   # [128, NI]

    NF = NI // 2  # two supertiles: overlap in-DMA / compute / out-DMA

    with TileContext(nc) as tc:
        with (
            tc.tile_pool(name="io", bufs=2) as io,
            tc.tile_pool(name="sc", bufs=2) as sc,
        ):
            for st in range(NI // NF):
                off = st * NF
                xin = io.tile([P, NF * D_IN], I32, tag="xin", name="xin")
                nc.sync.dma_start(xin[:, :], x_r[:, off * D_IN:(off + NF) * D_IN])
                xv = xin[:, :].rearrange("p (n d) -> p n d", d=D_IN)
                hi = xv[:, :, 0]
                lo = xv[:, :, 1]

                def T(tag):
                    t = sc.tile([P, NF], I32, tag=tag, name=tag)
                    return t[:, :]

                # All int32 ALU ops on vector (DVE): bitwise int32 is
                # DVE-only, and Pool-engine int ops measured ~14us each
                # (Q7 overhead + cross-engine sync), 3x worse overall.
                V = G = nc.vector

                # field extraction
                E = T("E")          # 11-bit biased fp64 exponent
                V.tensor_scalar(E, hi, 20, 0x7FF,
                                AOT.logical_shift_right, AOT.bitwise_and)
                Mhi = T("Mhi")
                V.tensor_scalar(Mhi, hi, 0xFFFFF, 3,
                                AOT.bitwise_and, AOT.logical_shift_left)
                Mlo = T("Mlo")
                V.tensor_scalar(Mlo, lo, 29, 7,
                                AOT.logical_shift_right, AOT.bitwise_and)
                M23 = T("M23")      # top 23 mantissa bits as an int
                V.tensor_tensor(M23, Mhi, Mlo, AOT.bitwise_or)
                R = T("R")          # round bit (mant bit 23)
                V.tensor_scalar(R, lo, 28, 1,
                                AOT.logical_shift_right, AOT.bitwise_and)
                sval = T("sval")    # sticky field (mant bits 24..51)
                V.tensor_scalar(sval, lo, 0x0FFFFFFF, None, AOT.bitwise_and)
                S = T("S")
                G.tensor_scalar(S, sval, 1, None, AOT.min)
                # round-to-nearest-even: round_up = R & (S | lsb)
                L = T("L")
                V.tensor_scalar(L, M23, 1, None, AOT.bitwise_and)
                SL = T("SL")
                V.tensor_tensor(SL, S, L, AOT.bitwise_or)
                ru = T("ru")
                V.tensor_tensor(ru, R, SL, AOT.bitwise_and)
                Mr = T("Mr")
                V.tensor_tensor(Mr, M23, ru, AOT.add)
                c_m = T("c_m")      # mantissa carry into the exponent
                V.tensor_scalar(c_m, Mr, 23, None, AOT.logical_shift_right)
                mant_f = T("mant_f")
                V.tensor_scalar(mant_f, Mr, 0x7FFFFF, None, AOT.bitwise_and)
                # rebias: newE = (E - 896) + c_m
                newE = T("newE")
                V.scalar_tensor_tensor(newE, E, -896, c_m, AOT.add, AOT.add)
                nsh = T("nsh")
                V.tensor_scalar(nsh, newE, 23, None, AOT.logical_shift_left)
                body = T("body")
                V.tensor_tensor(body, nsh, mant_f, AOT.bitwise_or)
                # specials (feeders, off the critical chain)
                over = T("over")
                G.tensor_scalar(over, E, 1151, None, AOT.is_ge)
                under = T("under")
                G.tensor_scalar(under, E, 897, None, AOT.is_lt)
                lml = T("lml")      # mant bits 23..51
                V.tensor_scalar(lml, lo, 0x1FFFFFFF, None, AOT.bitwise_and)
                manyv = T("manyv")
                V.tensor_tensor(manyv, M23, lml, AOT.bitwise_or)
                eq2047 = T("eq2047")
                G.tensor_scalar(eq2047, E, 2047, None, AOT.is_equal)
                many = T("many")
                G.tensor_scalar(many, manyv, 1, None, AOT.min)
                nan = T("nan")
                V.tensor_tensor(nan, eq2047, many, AOT.bitwise_and)
                om = T("om")
                G.tensor_scalar(om, over, 1, None, AOT.subtract)
                um = T("um")
                G.tensor_scalar(um, under, 1, None, AOT.subtract)
                nm = T("nm")
                G.tensor_scalar(nm, nan, 1, None, AOT.subtract)
                sb = T("sb")
                V.tensor_scalar(sb, hi, 31, 31,
                                AOT.logical_shift_right, AOT.logical_shift_left)
                # body1 = over ? 0x7F800000 : body   (xor/and with NOT-mask)
                x1 = T("x1")
                V.tensor_scalar(x1, body, 0x7F800000, None, AOT.bitwise_xor)
                x2 = T("x2")
                V.tensor_tensor(x2, x1, om, AOT.bitwise_and)
                body1 = T("body1")
                V.tensor_scalar(body1, x2, 0x7F800000, None, AOT.bitwise_xor)
                # body2 = under ? 0 : body1
                body2 = T("body2")
                V.tensor_tensor(body2, body1, um, AOT.bitwise_and)
                # body3 = nan ? 0x7FC00000 : body2
                x3 = T("x3")
                V.tensor_scalar(x3, body2, 0x7FC00000, None, AOT.bitwise_xor)
                x4 = T("x4")
                V.tensor_tensor(x4, x3, nm, AOT.bitwise_and)
                body3 = T("body3")
                V.tensor_scalar(body3, x4, 0x7FC00000, None, AOT.bitwise_xor)
                yt = io.tile([P, NF], I32, tag="yt", name="yt")
                V.tensor_tensor(yt[:, :], body3, sb, AOT.bitwise_or)
                nc.sync.dma_start(y_r[:, off:off + NF], yt[:, :])

    nc.compile()
    return nc


# ---------------- host-side pack / unpack (XLA CPU, multithreaded) ----------
_PACK_W = (np.uint32(1) << np.arange(31, -1, -1, dtype=np.uint32)).astype(np.int32)


def _pack_cpu(xf):
    # {0.,1.} float bits, MSB-first -> int32 words; int32 add-wrap == OR here
    xi = xf.astype(jnp.int32).reshape(-1, D_IN, 32)
    return (xi * _PACK_W[None, None, :]).sum(axis=-1, dtype=jnp.int32)


def _unpack_cpu(w):
    sh = jnp.arange(31, -1, -1, dtype=jnp.int32)
    bits = jnp.right_shift(w.reshape(-1, 1).view(jnp.uint32),
                           sh.view(jnp.uint32)[None, :]) & jnp.uint32(1)
    return bits.astype(jnp.float32)


def _pack_input_np(x: np.ndarray) -> np.ndarray:
    xp = np.packbits(x != 0, axis=-1)
    return xp.view(np.dtype(">u4")).astype(np.uint32).view(np.int32)


def _unpack_output_np(w: np.ndarray) -> np.ndarray:
    wbe = w.view(np.uint32).astype(np.dtype(">u4"))
    bits = np.unpackbits(wbe.view(np.uint8).reshape(-1, 4), axis=-1)
    return bits.astype(np.float32)


# ---------------- cached executor ----------------
_STATE: dict = {}
_LOCK = threading.Lock()


def _prepare_locked():
    if "ready" in _STATE or "failed" in _STATE:
        return
    try:
        nc = _build()
        _STATE["nc"] = nc

        # official path first: compile + run the Bass kernel via
        # run_bass_kernel_spmd (dummy input); also warms devices + NEFF.
        dummy = np.zeros((B_CORE, D_IN), np.int32)
        in_maps = [{"x": dummy} for _ in range(N_CORES)]
        res = run_bass_kernel_spmd(nc, in_maps, core_ids=list(range(N_CORES)))
        w_official = np.concatenate([r["y"] for r in res.results], axis=0)

        pack_jit = jax.jit(_pack_cpu, backend="cpu")
        unpack_jit = jax.jit(_unpack_cpu, backend="cpu")

        bass2jax.install_neuronx_cc_hook()
        pn = nc.partition_id_tensor.name if nc.partition_id_tensor else None
        in_names, out_names, out_avals = [], [], []
        for alloc in nc.m.functions[0].allocations:
            if not isinstance(alloc, mybir.MemoryLocationSet):
                continue
            name = alloc.memorylocations[0].name
            if alloc.kind == "ExternalInput":
                if name != pn:
                    in_names.append(name)
            elif alloc.kind == "ExternalOutput":
                out_names.append(name)
                out_avals.append(jax.core.ShapedArray(
                    tuple(alloc.tensor_shape), mybir.dt.np(alloc.dtype)))
        n_params, n_outs = len(in_names), len(out_avals)
        in_names_all = in_names + out_names + ([pn] if pn else [])
        donate = tuple(range(n_params, n_params + n_outs))

        def _body(*args):
            operands = list(args)
            if pn is not None:
                operands.append(bass2jax.partition_id_tensor())
            return tuple(bass2jax._bass_exec_p.bind(
                *operands, out_avals=tuple(out_avals),
                in_names=tuple(in_names_all), out_names=tuple(out_names),
                lowering_input_output_aliases=(),
                sim_require_finite=True, sim_require_nnan=True, nc=nc))

        devices = jax.devices()[:N_CORES]
        mesh = Mesh(np.asarray(devices), ("core",))
        spec = PartitionSpec("core")
        shd = NamedSharding(mesh, spec)
        sharded = jax.jit(
            shard_map(_body, mesh=mesh, in_specs=(spec,) * (n_params + n_outs),
                      out_specs=(spec,) * n_outs, check_rep=False),
            donate_argnums=donate, keep_unused=True)
        g_out = (N_CORES * out_avals[0].shape[0], *out_avals[0].shape[1:])
        zeros_jit = jax.jit(lambda: jnp.zeros(g_out, out_avals[0].dtype),
                            out_shardings=shd)

        # warm-compile + cross-check the fast path against the official run
        xg = np.zeros((B, D_IN), np.int32)
        out = sharded(xg, zeros_jit())
        w_fast = np.asarray(out[0])
        assert np.array_equal(w_fast, w_official), "fast path mismatch"
        # warm the host pack/unpack jits too
        pack_jit(np.zeros((4096, 64), np.float32))
        unpack_jit(np.zeros((4096, 1), np.int32))

        _STATE.update(dict(pack_jit=pack_jit, unpack_jit=unpack_jit,
                           sharded=sharded, zeros_jit=zeros_jit, ready=True))
    except Exception as e:  # fall back to the plain spmd path per call
        _STATE["failed"] = repr(e)
        if "nc" not in _STATE:
            _STATE["nc"] = _build()


def _prepare():
    with _LOCK:
        _prepare_locked()


def _get_nc():
    _prepare()
    return _STATE["nc"]


_WARM = threading.Thread(target=_prepare, daemon=True)
_WARM.start()


def kernel(fp64_pulse: np.ndarray) -> np.ndarray:
    x = np.asarray(fp64_pulse)
    assert x.shape == (B, 64)
    _prepare()
    if "ready" in _STATE:
        zeros = _STATE["zeros_jit"]()                    # async, on-device
        xw = np.asarray(_STATE["pack_jit"](x))           # (B, 2) int32
        out = _STATE["sharded"](xw, zeros)
        w = np.asarray(out[0])                           # (B, 1) int32
        return np.asarray(_STATE["unpack_jit"](w))
    # fallback: plain official path with numpy pack/unpack
    nc = _STATE["nc"]
    xw = _pack_input_np(x)
    in_maps = [{"x": xw[c * B_CORE:(c + 1) * B_CORE]} for c in range(N_CORES)]
    res = run_bass_kernel_spmd(nc, in_maps, core_ids=list(range(N_CORES)))
    w = np.concatenate([r["y"] for r in res.results], axis=0)
    return _unpack_output_np(w)
